# revision 20
# baseline (speedup 1.0000x reference)
"""Trainium2 Bass kernel for nn_GATModel (2-layer GAT + mean-pool + MLP head).

v3 strategy (8 NeuronCores, SPMD, dst-sharded):
  - Edges sorted by dst; each core owns a contiguous 6250-node range and all
    edges pointing into it. 49 windows of 128 dst nodes; per-window chunk
    count k_w = max over cores (shared SPMD program shape).
  - Per-node table rows gathered per edge via batched dma_gather (256B rows).
    table1: bf16 [z1(102)|el|er|pad]; table2: fp8 z2(198) + bf16 el/er at
    byte 200 (256B rows, half the v2 bytes).
  - Tables AllGathered in window GROUPS (K_GW windows per collective) so the
    collective overlaps table construction (t2 groups fire inside the L1
    edge loop).
  - Edge phase per window: batched indirect gather; ONE merged mask DMA
    (maskT|wm0 fp8); e=el+er via DVE; lrelu via scalar_tensor_tensor;
    exp on ACT; scaling via 3 stride-0 broadcast multiplies per window;
    per-chunk aggregation matmul accumulating in PSUM.
  - Mean-pool fused into layer-2 epilogue, AllReduce [192,128], dense head.
"""
import math
import os
from contextlib import ExitStack

import numpy as np
import ml_dtypes

import concourse.bacc as bacc
import concourse.bass as bass
import concourse.tile as tile
from concourse import mybir
from concourse.bass_utils import run_bass_kernel_spmd

dt = mybir.dt

N_NODES = 50000
N_EDGES = 800000
N_GRAPHS = 128
NEG = 0.2
NC = 8
NPC = N_NODES // NC            # 6250 nodes per core
NWIN = math.ceil(NPC / 128)    # 49 windows per core
H = 3
D1, D2 = 32, 64
B1 = D1 + 2                    # L1 head block (bf16 cols): z | one | pad
B2 = D2 + 2                    # L2 head block (fp8 cols)
ZC1 = H * B1                   # 102 bf16 cols
ZC2 = H * B2                   # 198 fp8 cols
R1 = 128                       # L1 row: 128 bf16 cols = 256B
R2 = 256                       # L2 row: 256 fp8 cols = 256B
EL1 = ZC1                      # L1 el at bf16 col 102 (er at 105)
EL2V = 100                     # L2 el at bf16 col 100 of bitcast view (byte 200)
NLO = 32768                    # int16 index limit for lo-section gathers
X1C, X2C = 96, 192

_CACHE = {}


def _ceil(a, b):
    return (a + b - 1) // b


def _grouping():
    GW = int(os.environ.get("K_GW", "99"))
    groups = []
    w = 0
    while w < NWIN:
        w1 = min(w + GW, NWIN)
        r0 = w * 128
        r1 = min(w1 * 128, NPC)
        groups.append((r0, r1))
        w = w1
    return groups


def build_program(kws):
    """kws: per-window (lo, hi) chunk counts (shared across cores)."""
    STAGE = int(os.environ.get("K_STAGE", "99"))
    NW_LIM = int(os.environ.get("K_NWIN", str(NWIN)))
    REP = int(os.environ.get("K_REP", "1"))
    groups = _grouping()
    SCRATCH = int(os.environ.get("K_SCRATCH", "49152"))
    nc = bacc.Bacc("TRN2", target_bir_lowering=False, debug=False, num_devices=NC,
                   dynamic_dma_scratch_size=SCRATCH, num_swdge_queues=4)
    kws = list(kws)              # [(kA, kB)] per window
    ktot = [a + b for a, b in kws]
    TCH = sum(ktot)
    K0s = np.concatenate([[0], np.cumsum(ktot)]).astype(int)
    ECOLS = TCH * 128

    # ---------------- I/O ----------------
    featT = nc.dram_tensor("featT", [11, NPC], dt.float32, kind="ExternalInput").ap()
    wcat1 = nc.dram_tensor("wcat1", [11, R1], dt.float32, kind="ExternalInput").ap()
    wcat2 = nc.dram_tensor("wcat2", [X1C + 1, 204], dt.bfloat16, kind="ExternalInput").ap()
    idx_in = nc.dram_tensor("idx16", [128, 8 * TCH], dt.int16, kind="ExternalInput").ap()
    mask2_in = nc.dram_tensor("mask2", [128, 2 * ECOLS], dt.float8e4, kind="ExternalInput").ap()
    gidc_in = nc.dram_tensor("gidc", [128, NWIN], dt.float32, kind="ExternalInput").ap()
    invc_in = nc.dram_tensor("invc", [128, NWIN], dt.float32, kind="ExternalInput").ap()
    d1a_in = nc.dram_tensor("d1a", [128, 64], dt.float32, kind="ExternalInput").ap()
    d1b_in = nc.dram_tensor("d1b", [65, 64], dt.float32, kind="ExternalInput").ap()
    d2_in = nc.dram_tensor("d2", [65, 1], dt.float32, kind="ExternalInput").ap()
    ident_in = nc.dram_tensor("ident", [128, 128], dt.bfloat16, kind="ExternalInput").ap()
    iota_in = nc.dram_tensor("iota_row", [128, 128], dt.bfloat16, kind="ExternalInput").ap()
    out_ext = nc.dram_tensor("out", [N_GRAPHS, 1], dt.float32, kind="ExternalOutput").ap()

    rg = [list(range(NC))]

    with tile.TileContext(nc) as tc, ExitStack() as ctx:
        cst = ctx.enter_context(tc.tile_pool(name="cst", bufs=1))
        sb = ctx.enter_context(tc.tile_pool(name="sb", bufs=2))
        dr = ctx.enter_context(tc.tile_pool(name="dr", bufs=1, space="DRAM"))
        psA = ctx.enter_context(tc.tile_pool(name="psA", bufs=2, space="PSUM"))

        # ---------------- constants / resident ----------------
        ident = cst.tile([128, 128], dt.bfloat16)
        nc.sync.dma_start(out=ident[:], in_=ident_in)
        iota_row = cst.tile([128, 128], dt.bfloat16)
        nc.sync.dma_start(out=iota_row[:], in_=iota_in)

        idx16 = cst.tile([128, 8 * TCH], dt.int16)
        nc.sync.dma_start(out=idx16[:], in_=idx_in)
        gidc = cst.tile([128, NWIN], dt.float32)
        nc.sync.dma_start(out=gidc[:], in_=gidc_in)
        invc = cst.tile([128, NWIN], dt.float32)
        nc.sync.dma_start(out=invc[:], in_=invc_in)
        w1sb = cst.tile([11, R1], dt.float32)
        nc.sync.dma_start(out=w1sb[:], in_=wcat1)
        w2sb = cst.tile([X1C + 1, 204], dt.bfloat16)
        nc.sync.dma_start(out=w2sb[:], in_=wcat2)
        ftsb = cst.tile([11, NPC], dt.float32)
        nc.sync.dma_start(out=ftsb[:], in_=featT)
        d1a = cst.tile([128, 64], dt.float32)
        nc.sync.dma_start(out=d1a[:], in_=d1a_in)
        d1b = cst.tile([65, 64], dt.float32)
        nc.sync.dma_start(out=d1b[:], in_=d1b_in)
        d2w = cst.tile([65, 1], dt.float32)
        nc.sync.dma_start(out=d2w[:], in_=d2_in)

        er1_sb = cst.tile([128, 4 * NWIN], dt.bfloat16)
        nc.vector.memset(er1_sb[:], 0.0)
        er2_sb = cst.tile([128, 4 * NWIN], dt.bfloat16)
        nc.vector.memset(er2_sb[:], 0.0)

        t1_shard = dr.tile([NPC, R1], dt.bfloat16)
        t2_shard = dr.tile([NPC, R2], dt.float8e4)
        pool_loc = dr.tile([X2C, N_GRAPHS], dt.float32)

        for rep in range(REP):
          table1_t = dr.tile([N_NODES, R1], dt.bfloat16, addr_space="Shared",
                             name=f"table1_t{rep}")
          table2_t = dr.tile([N_NODES, R2], dt.float8e4, addr_space="Shared",
                             name=f"table2_t{rep}")
          pool_red_t = dr.tile([X2C, N_GRAPHS], dt.float32, addr_space="Shared",
                               name=f"pool_red_t{rep}")

          def ag_group(shard, table_t, r0, r1):
              nc.gpsimd.collective_compute(
                  "AllGather", mybir.AluOpType.bypass, replica_groups=rg,
                  ins=[shard[r0:r1, :]],
                  outs=[table_t[NC * r0:NC * r0 + NC * (r1 - r0), :]])

          with tc.tile_pool(name=f"psB{rep}", bufs=2, space="PSUM") as psB:
            # phase 0: layer-1 table shard (+ grouped AllGather)
            gi = 0
            for w in range(NWIN):
                n0 = w * 128
                nw = min(128, NPC - n0)
                zpb = psB.tile([128, 204], dt.float32, tag="zbig", name="zp")
                zp = zpb[:, 0:R1]
                nc.tensor.matmul(zp[:nw, :], lhsT=ftsb[:, n0:n0 + nw], rhs=w1sb[:],
                                 start=True, stop=True)
                zb = sb.tile([128, R1], dt.bfloat16, tag="zb1", name="zb1")
                nc.vector.tensor_copy(zb[:nw, :], zp[:nw, :])
                nc.vector.tensor_copy(er1_sb[:nw, 4 * w:4 * w + 3],
                                      zp[:nw, EL1 + 3:EL1 + 6])
                nc.sync.dma_start(out=t1_shard[n0:n0 + nw, :], in_=zb[:nw, :])
                if gi < len(groups) and n0 + nw >= groups[gi][1]:
                    ag_group(t1_shard, table1_t, *groups[gi])
                    gi += 1

            if STAGE >= 1:
                edge_layer(nc, tc, sb, psA, psB, None, 1,
                           kws[:NW_LIM], K0s,
                           table1_t, idx16, mask2_in, er1_sb, er2_sb,
                           ident, iota_row, gidc, invc, w2sb, t2_shard,
                           table2_t, groups, ag_group)

          if STAGE < 3:
            osb0 = sb.tile([N_GRAPHS, 1], dt.float32, tag="osb", name="osb0")
            nc.vector.memset(osb0[:, :], 0.0)
            nc.sync.dma_start(out=out_ext, in_=osb0[:, :])

          if STAGE >= 3:
            with tc.tile_pool(name=f"psC{rep}", bufs=1, space="PSUM") as psC:
              pa, pb = edge_layer(nc, tc, sb, psA, None, psC, 2,
                                  kws[:NW_LIM], K0s,
                                  table2_t, idx16, mask2_in, er2_sb, None,
                                  ident, iota_row, gidc, invc, None, None,
                                  None, None, None)

              # ---------------- pooling reduce + head ----------------
              pasb = sb.tile([128, N_GRAPHS], dt.float32, tag="pasb", name="pasb")
              nc.vector.tensor_copy(pasb[:, :], pa[:, :])
              pbsb = sb.tile([64, N_GRAPHS], dt.float32, tag="pbsb", name="pbsb")
              nc.vector.tensor_copy(pbsb[:, :], pb[:, :])
            nc.sync.dma_start(out=pool_loc[0:128, :], in_=pasb[:, :])
            nc.sync.dma_start(out=pool_loc[128:192, :], in_=pbsb[:, :])
            pra = sb.tile([128, N_GRAPHS], dt.float32, tag="pra", name="pra")
            prb = sb.tile([65, N_GRAPHS], dt.float32, tag="prb", name="prb")
            nc.gpsimd.collective_compute(
                "AllReduce", mybir.AluOpType.add, replica_groups=rg,
                ins=[pool_loc[:, :]], outs=[pool_red_t[:, :]])
            nc.sync.dma_start(out=pra[:, :], in_=pool_red_t[0:128, :])
            nc.sync.dma_start(out=prb[:64, :], in_=pool_red_t[128:192, :])
            nc.vector.memset(prb[64:, :], 1.0)

            u1 = psA.tile([64, N_GRAPHS], dt.float32, tag="erp", name="u1")
            nc.tensor.matmul(u1[:, :], lhsT=d1a[:, :], rhs=pra[:, :],
                             start=True, stop=False)
            nc.tensor.matmul(u1[:, :], lhsT=d1b[:, :], rhs=prb[:, :],
                             start=False, stop=True)
            h1 = sb.tile([65, N_GRAPHS], dt.float32, tag="h1", name="h1")
            nc.scalar.activation(h1[:64, :], u1[:, :],
                                 mybir.ActivationFunctionType.Relu)
            nc.vector.memset(h1[64:, :], 1.0)
            o_ps = psA.tile([N_GRAPHS, 1], dt.float32, tag="agg", name="ops")
            nc.tensor.matmul(o_ps[:, :], lhsT=h1[:, :], rhs=d2w[:, :],
                             start=True, stop=True)
            osb = sb.tile([N_GRAPHS, 1], dt.float32, tag="osb", name="osb")
            nc.vector.tensor_copy(osb[:, :], o_ps[:, :])
            nc.sync.dma_start(out=out_ext, in_=osb[:, :])

    nc.finalize()
    return nc


def edge_layer(nc, tc, sb, psA, psB, psC, layer, kws, K0s,
               tabl, idx16, mask2_in, er_sb, ernext_sb,
               ident, iota_row, gidc, invc, w2sb, t2_shard,
               table2_t, groups, ag_group):
    B = B1 if layer == 1 else B2
    ZC = ZC1 if layer == 1 else ZC2
    XC = X1C if layer == 1 else X2C
    KMAX = max(a + b for a, b in kws)
    eps = 1e-16
    WOP = int(os.environ.get("K_WOP", "9"))

    if layer == 2:
        pa = psC.tile([128, N_GRAPHS], dt.float32, tag="poolA", name="poolA")
        pb = psC.tile([64, N_GRAPHS], dt.float32, tag="poolB", name="poolB")
    gq = 0
    gi = 0
    state = {}
    state2 = {}

    def front(w):
        """gathers + mask DMA + erp + e/w chain + scale (window w)"""
        nonlocal gq
        kA, kB = kws[w]
        k = kA + kB
        K0 = int(K0s[w])

        # batched gathers: slot i of window -> zg[i%128, i//128, :]
        # lo section (chunks 0..kA): idx < NLO; hi section: idx-NLO
        zdt = dt.bfloat16 if layer == 1 else dt.float8e4
        R = R1 if layer == 1 else R2
        zg = sb.tile([128, KMAX, R], zdt, tag=f"zg{layer}", bufs=3, name="zg")
        # >1024 descriptors per dma_gather wedges the device at the default
        # 49152B SWDGE ring (1024 x 48B); K_SCRATCH scales the ring.
        GMAX = int(os.environ.get("K_GMAX", "8"))
        if not int(os.environ.get("K_NOGATHER", "0")):
            for base, koff, ksec in ((0, 0, kA), (NLO, kA, kB)):
                for g0 in range(0, ksec, GMAX):
                    gk = min(GMAX, ksec - g0)
                    c0 = koff + g0
                    nc.gpsimd.dma_gather(
                        out_ap=zg[:, c0:c0 + gk, :],
                        in_ap=tabl[base:, :] if base else tabl[:, :],
                        idxs_ap=idx16[:, 8 * (K0 + c0):8 * (K0 + c0 + gk)],
                        num_idxs=gk * 128, num_idxs_reg=gk * 128, elem_size=R,
                        queue_num=gq % 4)
                    gq += 1
        # el (+first er col) as bf16 view
        zgb = zg[:, :, :] if layer == 1 else zg[:, :, :].bitcast(dt.bfloat16)
        ELV = EL1 if layer == 1 else EL2V

        m2 = sb.tile([128, 2 * KMAX * 128], dt.float8e4, tag="m2", bufs=4,
                     name="m2")
        if WOP >= 1:
            nc.sync.dma_start(out=m2[:, 0:2 * k * 128],
                              in_=mask2_in[:, 2 * 128 * K0:2 * 128 * (K0 + k)])
        maskT = m2[:, 0:k * 128]
        wm0 = m2[:, k * 128:2 * k * 128]

        # er expand node->edge: erp[p,c,j] = er[dst(edge (c,p)), j]
        erp = psA.tile([128, KMAX, 4], dt.float32, tag="erp", name="erp")
        if WOP >= 2:
            for c in range(k):
                nc.tensor.matmul(erp[:, c, 0:4],
                                 lhsT=maskT[:, c * 128:(c + 1) * 128],
                                 rhs=er_sb[:, 4 * w:4 * w + 4],
                                 start=True, stop=True)
        # e = el + er ; w = exp(lrelu(e))
        wb = sb.tile([128, KMAX, 4], dt.float32, tag="wb", name="wb")
        if WOP >= 3:
            ebuf = sb.tile([128, KMAX, 4], dt.float32, tag="ebuf", name="ebuf")
            nc.vector.tensor_tensor(out=ebuf[:, 0:k, :], in0=erp[:, 0:k, :],
                                    in1=zgb[:, 0:k, ELV:ELV + 4],
                                    op=mybir.AluOpType.add)
            elr = sb.tile([128, KMAX, 4], dt.float32, tag="elr", name="elr")
            nc.vector.scalar_tensor_tensor(
                out=elr[:, 0:k, :], in0=ebuf[:, 0:k, :], scalar=NEG,
                in1=ebuf[:, 0:k, :], op0=mybir.AluOpType.mult,
                op1=mybir.AluOpType.max)
            nc.scalar.activation(wb[:, 0:k, :], elr[:, 0:k, :],
                                 mybir.ActivationFunctionType.Exp)
        else:
            nc.vector.memset(wb[:, :, :], 1.0)

        # scale gathered z blocks (incl ones col) by w:
        # one broadcast (stride-0) multiply per head over the whole window.
        # L1 scales bf16 in place; L2 reads fp8, writes bf16 zgs.
        if layer == 1:
            zsc = zg
        else:
            zsc = sb.tile([128, KMAX, ZC2], dt.bfloat16, tag="zgs", name="zgs")
        if WOP >= 4:
            for h in range(H):
                xs = zg[:, 0:k, h * B:(h + 1) * B]
                os_ = zsc[:, 0:k, h * B:(h + 1) * B]
                ws, _ = bass.broadcast_tensor_aps(wb[:, 0:k, h:h + 1], xs)
                nc.vector.tensor_tensor(out=os_, in0=xs, in1=ws,
                                        op=mybir.AluOpType.mult)
        state[w] = (k, wm0, zsc)

    def back1(w):
        """aggregation + epilogue -> xsb (window w)"""
        k, wm0, zsc = state.pop(w)
        agg = psA.tile([128, H, B], dt.float32, tag="agg", name="agg")
        if WOP >= 5:
            for c in range(k):
                nc.tensor.matmul(agg[:, :, :], lhsT=wm0[:, c * 128:(c + 1) * 128],
                                 rhs=zsc[:, c, 0:ZC],
                                 start=(c == 0), stop=(c == k - 1))

        xsb = sb.tile([128, XC], dt.bfloat16, tag="xsb", bufs=3, name="xsb")
        if WOP >= 6:
            # epilogue: x = relu(agg_z / s) (* 1/cnt for layer 2)
            D = D1 if layer == 1 else D2
            seps = sb.tile([128, H], dt.float32, tag="seps", name="seps")
            nc.vector.tensor_scalar(out=seps[:, 0:H],
                                    in0=agg[:, :, D:D + 1],
                                    scalar1=eps, scalar2=None,
                                    op0=mybir.AluOpType.add)
            invs = sb.tile([128, H], dt.float32, tag="invs", name="invs")
            nc.vector.reciprocal(invs[:, :], seps[:, :])
            if layer == 2:
                nc.vector.tensor_scalar(out=invs[:, :], in0=invs[:, :],
                                        scalar1=invc[:, w:w + 1], scalar2=None,
                                        op0=mybir.AluOpType.mult)
            for h in range(H):
                nc.scalar.activation(xsb[:, h * D:(h + 1) * D],
                                     agg[:, h, 0:D],
                                     mybir.ActivationFunctionType.Relu,
                                     scale=invs[:, h:h + 1])
        else:
            nc.vector.memset(xsb[:, :], 0.01)
        state2[w] = xsb

    def back2(w):
        """z2-row build / pool accumulation (window w)"""
        nonlocal gi
        xsb = state2.pop(w)
        n0 = w * 128
        nw = min(128, NPC - n0)
        if layer == 1 and WOP >= 7:
            # transpose x1 -> z2 rows (fp8 z + bf16 el/er) -> t2 shard
            xtp = psB.tile([X1C, 128], dt.bfloat16, tag="xtp", name="xtp")
            nc.tensor.transpose(xtp[:, :], xsb[:, :], ident[:])
            xta = sb.tile([X1C + 1, 128], dt.bfloat16, tag="xta", name="xta")
            nc.vector.tensor_copy(xta[:X1C, :], xtp[:, :])
            nc.vector.memset(xta[X1C:, :], 1.0)
            z2p = psB.tile([128, 204], dt.float32, tag="zbig", name="z2p")
            nc.tensor.matmul(z2p[:, :], lhsT=xta[:, :], rhs=w2sb[:],
                             start=True, stop=True)
            z2b = sb.tile([128, R2], dt.float8e4, tag="z2b", name="z2b")
            nc.vector.tensor_copy(z2b[:, 0:ZC2], z2p[:, 0:ZC2])
            z2bb = z2b[:, :].bitcast(dt.bfloat16)
            nc.vector.tensor_copy(z2bb[:, EL2V:EL2V + 6], z2p[:, ZC2:ZC2 + 6])
            nc.vector.tensor_copy(ernext_sb[:nw, 4 * w:4 * w + 3],
                                  z2p[:nw, ZC2 + 3:ZC2 + 6])
            nc.sync.dma_start(out=t2_shard[n0:n0 + nw, :], in_=z2b[:nw, :])
            if gi < len(groups) and n0 + nw >= groups[gi][1]:
                ag_group(t2_shard, table2_t, *groups[gi])
                gi += 1
        elif layer == 2 and WOP >= 7:
            pm = sb.tile([128, N_GRAPHS], dt.bfloat16, tag="pm", name="pm")
            nc.vector.tensor_scalar(out=pm[:], in0=iota_row[:],
                                    scalar1=gidc[:, w:w + 1], scalar2=None,
                                    op0=mybir.AluOpType.is_equal)
            nc.tensor.matmul(pa[:, :], lhsT=xsb[:, 0:128], rhs=pm[:],
                             start=(w == 0), stop=(w == len(kws) - 1))
            nc.tensor.matmul(pb[:, :], lhsT=xsb[:, 128:192], rhs=pm[:],
                             start=(w == 0), stop=(w == len(kws) - 1))

    STAG = int(os.environ.get("K_STAG", "1"))
    STAG2 = STAG + int(os.environ.get("K_STAG2", "1"))
    nw_tot = len(kws)
    for w in range(nw_tot + STAG2):
        if w < nw_tot:
            front(w)
        if STAG <= w < nw_tot + STAG:
            back1(w - STAG)
        if w >= STAG2:
            back2(w - STAG2)
    if layer == 2:
        return pa, pb


# ======================= host side =======================

def _prep(feature, src, dst, graph_ids, W1, al1, ar1, W2, al2, ar2,
          d1_w, d1_b, d2_w, d2_b):
    feature = np.asarray(feature, np.float32)
    src = np.asarray(src, np.int64)
    dst = np.asarray(dst, np.int64)
    graph_ids = np.asarray(graph_ids, np.int64)

    order = np.argsort(dst, kind="stable")
    src_s = src[order].astype(np.int64)
    dst_s = dst[order].astype(np.int64)

    cnts = np.bincount(graph_ids, minlength=N_GRAPHS).astype(np.float32)
    cnts = np.maximum(cnts, 1.0)
    node_inv = (1.0 / cnts)[graph_ids]

    # grouped-table row remap: node n -> gathered-table row index
    groups = _grouping()
    goff = np.zeros(NPC, np.int64)     # per-local-row: group local start
    rows_g = np.zeros(NPC, np.int64)   # per-local-row: rows in its group
    gbase = np.zeros(NPC, np.int64)    # per-local-row: table base of group
    base = 0
    for (r0, r1) in groups:
        goff[r0:r1] = r0
        rows_g[r0:r1] = r1 - r0
        gbase[r0:r1] = base
        base += NC * (r1 - r0)
    n_all = np.arange(N_NODES, dtype=np.int64)
    c_all = n_all // NPC
    r_all = n_all % NPC
    remap = gbase[r_all] + c_all * rows_g[r_all] + (r_all - goff[r_all])
    src_g = remap[src_s]

    # window boundaries; per-window lo/hi chunk counts = max over cores
    percore = []
    kAs = np.zeros(NWIN, np.int64)
    kBs = np.zeros(NWIN, np.int64)
    for r in range(NC):
        wins = []
        for w in range(NWIN):
            lo = r * NPC + w * 128
            hi = min(r * NPC + NPC, lo + 128)
            e0 = np.searchsorted(dst_s, lo, side="left")
            e1 = np.searchsorted(dst_s, hi, side="left")
            m = int((src_g[e0:e1] < NLO).sum())
            q = (e1 - e0) - m
            wins.append((lo, hi, e0, e1))
            kAs[w] = max(kAs[w], _ceil(m, 128))
            kBs[w] = max(kBs[w], _ceil(q, 128))
        percore.append(wins)
    kAs = np.maximum(kAs, 1)  # keep >=1 chunk so every window aggregates
    kws = tuple((int(a), int(b)) for a, b in zip(kAs, kBs))
    ktot = [a + b for a, b in kws]
    TCH = sum(ktot)
    K0s = np.concatenate([[0], np.cumsum(ktot)]).astype(int)
    ECOLS = TCH * 128

    # weight prep
    W1 = np.asarray(W1, np.float32); W2 = np.asarray(W2, np.float32)
    al1 = np.asarray(al1, np.float32); ar1 = np.asarray(ar1, np.float32)
    al2 = np.asarray(al2, np.float32); ar2 = np.asarray(ar2, np.float32)

    def wcat(W, al, ar, D, B, ZC, R):
        F = W.shape[0]
        A_l = np.zeros((H * D, H), np.float32)
        A_r = np.zeros((H * D, H), np.float32)
        for h in range(H):
            A_l[h * D:(h + 1) * D, h] = al[h]
            A_r[h * D:(h + 1) * D, h] = ar[h]
        Wl = W @ A_l
        Wr = W @ A_r
        out = np.zeros((F + 1, R), np.float32)
        for h in range(H):
            out[:F, h * B:h * B + D] = W[:, h * D:(h + 1) * D]
            out[F, h * B + D] = 1.0          # ones column
        out[:F, ZC:ZC + 3] = Wl
        out[:F, ZC + 3:ZC + 6] = Wr
        return out

    wcat1 = wcat(W1, al1, ar1, D1, B1, ZC1, R1)
    wcat2 = wcat(W2, al2, ar2, D2, B2, ZC2, 204).astype(ml_dtypes.bfloat16)

    d1_w = np.asarray(d1_w, np.float32); d1_b = np.asarray(d1_b, np.float32)
    d2_w = np.asarray(d2_w, np.float32); d2_b = np.asarray(d2_b, np.float32)
    d1a = d1_w[0:128, :].copy()
    d1b = np.vstack([d1_w[128:192, :], d1_b[None, :]]).astype(np.float32)
    d2a = np.vstack([d2_w, d2_b[None, :]]).astype(np.float32)

    featT_all = feature.T.astype(np.float32)
    ONE = np.uint8(0x38)  # 1.0 in float8_e4m3

    in_maps = []
    for r in range(NC):
        idx16 = np.zeros((16, 8 * TCH), np.int16)
        mask2 = np.zeros((128, 2 * ECOLS), np.uint8)
        for w, (lo, hi, e0, e1) in enumerate(percore[r]):
            kA, kB = kws[w]
            K0 = int(K0s[w])
            es = src_g[e0:e1]
            ed = dst_s[e0:e1] - lo
            is_lo = es < NLO
            for sec, (sel, base2, koff, ksec) in enumerate([
                    (is_lo, 0, 0, kA), (~is_lo, NLO, kA, kB)]):
                s = es[sel] - base2
                dv = ed[sel].astype(np.int64)
                if int(os.environ.get("K_SORT", "1")):
                    so = np.argsort(s, kind="stable")
                    s, dv = s[so], dv[so]
                cnt = len(s)
                if ksec == 0:
                    assert cnt == 0
                    continue
                sv = np.zeros(ksec * 128, np.int16)
                sv[:cnt] = s.astype(np.int16)
                c0 = 8 * (K0 + koff)
                idx16[:, c0:c0 + 8 * ksec] = sv.reshape(-1, 16).T
                i = np.arange(cnt)
                ch = K0 + koff + i // 128
                # maskT[dst, slot] at col 2*128*K0 + (ch-K0)*... merged layout:
                # window w cols [2*128*K0, 2*128*(K0+k)): first k*128 maskT,
                # then k*128 wm0 (chunk-major within each).
                mbase = 2 * 128 * K0
                mask2[dv, mbase + 128 * (K0 + koff - K0) + i] = ONE
                wbase = mbase + 128 * (kA + kB)
                mask2[i % 128, wbase + 128 * (koff + i // 128) + dv] = ONE
        gidc = np.full((128, NWIN), -1.0, np.float32)
        invc = np.zeros((128, NWIN), np.float32)
        for w in range(NWIN):
            lo, hi, _, _ = percore[r][w]
            nw = hi - lo
            gidc[:nw, w] = graph_ids[lo:hi].astype(np.float32)
            invc[:nw, w] = node_inv[lo:hi]
        ft = np.vstack([featT_all[:, r * NPC:(r + 1) * NPC],
                        np.ones((1, NPC), np.float32)])
        in_maps.append({
            "featT": ft, "wcat1": wcat1, "wcat2": wcat2,
            "idx16": np.tile(idx16, (8, 1)),
            "mask2": mask2.view(ml_dtypes.float8_e4m3),
            "gidc": gidc, "invc": invc,
            "d1a": d1a, "d1b": d1b, "d2": d2a,
            "ident": np.eye(128, dtype=ml_dtypes.bfloat16),
            "iota_row": np.tile(np.arange(128, dtype=ml_dtypes.bfloat16)[None, :],
                                (128, 1)),
        })
    return in_maps, kws


def kernel(**inputs):
    in_maps, kws = _prep(**inputs)
    if kws not in _CACHE:
        _CACHE[kws] = build_program(kws)
    nc = _CACHE[kws]
    res = run_bass_kernel_spmd(nc, in_maps, list(range(NC)))
    return res.results[0]["out"]
